# revision 37
# baseline (speedup 1.0000x reference)
"""Trainium2 Bass kernel for nn_CGSC_64914135712264.

Sharding: 8 cores = (batch b in 0..3) x (H-half in 0..1). Each core computes
a [C=128, 28, 56] output slab of its batch. All params replicated.

v2 design (restructured from v1 for engine balance):
  - fus_w folded into the value conv (V' = (fus_w@wv)@x_skip); the bias term
    rides a rank-1 accumulating matmul with SD = sum_k dynw[k].  The 49-tap
    dynamic conv then produces x_fused directly.
  - q never materialized: scores = (wq^T key)^T @ x_skip (KW folding).
  - Per tap: PE broadcasts dynw row (PSUM) -> ACT casts to bf16 SBUF ->
    DVE does bf16 2x mul + add into one of 4 accumulator chains.  A subset
    of taps runs mul+add on GPSIMD into its own chain.
  - lepe diag-matmuls are interleaved into the PE stream as filler behind
    the broadcasts so the PE stays dense and HAM-warm.
  - 1/Z via ACT exp(-ln(Z)) instead of the 8-pass DVE reciprocal.
  - Channel attention: one AllGather (pair groups) instead of two serial
    AllReduces, with a dummy warm-up collective issued at kernel start to
    absorb the ~30us cold-start of the CC path.
"""

import sys

sys.path.insert(0, "/opt/trn_rl_repo")

import numpy as np
import ml_dtypes

import concourse.bass as bass
import concourse.mybir as mybir
import concourse.tile as tile
from concourse import bacc
from concourse.bass_utils import run_bass_kernel_spmd

BF16 = ml_dtypes.bfloat16
F32 = mybir.dt.float32
BF = mybir.dt.bfloat16

B, C, H, W = 4, 128, 56, 56
K = 7
K2 = 49
HALF = 28          # rows per core
HP, WP = HALF + 6, W + 6   # padded tile 34 x 62
N = HALF * W       # 1568 free elems per core
NCHUNK = 4
ROWS_PER_CHUNK = 7
CH_N = ROWS_PER_CHUNK * W  # 392
HN = 2 * CH_N              # 784 (half-tap free size, 14 rows)
SCALE = float(C) ** -0.5

# GPSIMD shares the DVE SBUF port (exclusive lock) -- offloading elementwise
# taps to it stalls the DVE, so everything elementwise stays on the DVE.
GP_TAPS = frozenset()

_CACHE = {}


def _build_program(use_collectives=True):
    nc = bacc.Bacc("TRN2", target_bir_lowering=False, debug=False, num_devices=8)
    f32, bf = F32, BF

    # ---- DRAM I/O (declaration order ~= DMA priority) ----
    d_xup = nc.dram_tensor("x_up", [C, H, W], bf, kind="ExternalInput")
    d_xpad = nc.dram_tensor("x_pad", [C, HP, WP], bf, kind="ExternalInput")
    # bf16 pack: wkT | wq | wvfT | wgT | ident | wwT(pad128) | ones49 | ones1_49 | bq
    d_wpack = nc.dram_tensor("wpack", [C, 5 * C + 2 * K2 + 2], bf, kind="ExternalInput")
    # f32 pack: bvf | bk | biasf | gbs | gbb | bw(pad128) | caw1T | caw2T(pad128)
    d_fpack = nc.dram_tensor("fpack", [C, 6 + 8 + C], f32, kind="ExternalInput")
    d_ldiag = nc.dram_tensor("lepe_diag", [C, K2, C], bf, kind="ExternalInput")
    d_xid = nc.dram_tensor("x_id", [C, N], f32, kind="ExternalInput")
    d_y = nc.dram_tensor("y", [C, N], f32, kind="ExternalOutput")
    d_dynw = nc.dram_tensor("dynw_hbm", [K2, N], bf)

    # collectives scratch
    cc_warm_in = nc.dram_tensor("cc_warm_in", [1, 8], f32)
    cc_warm_out = nc.dram_tensor("cc_warm_out", [2, 8], f32)
    cc_in = nc.dram_tensor("cc_in", [C, 2], f32)
    cc_out = nc.dram_tensor("cc_out", [2 * C, 2], f32)
    groups = [[0, 1], [2, 3], [4, 5], [6, 7]]

    AF = mybir.ActivationFunctionType
    ALU = mybir.AluOpType
    AX = mybir.AxisListType

    with tile.TileContext(nc, trace_sim=False) as tc:
        with (
            tc.tile_pool(name="const", bufs=1) as constp,
            tc.tile_pool(name="big", bufs=1) as bigp,
            tc.tile_pool(name="work", bufs=2) as workp,
            tc.tile_pool(name="pipe", bufs=3) as pipep,
            tc.tile_pool(name="lepe_ps", bufs=1, space="PSUM") as lepep,
        ):
            def load(pool, dram, shape, dtype):
                t = pool.tile(shape, dtype, tag=dram.name)
                nc.sync.dma_start(out=t[:], in_=dram[:])
                return t

            # warm up the CC path early (result unused)
            if use_collectives:
                nc.gpsimd.collective_compute(
                    "AllGather", ALU.bypass, replica_groups=groups,
                    ins=[cc_warm_in[:]], outs=[cc_warm_out[:]],
                )

            xup = load(bigp, d_xup, [C, H, W], bf)
            wpack = load(constp, d_wpack, [C, 5 * C + 2 * K2 + 2], bf)
            fpack = load(constp, d_fpack, [C, 6 + 8 + C], f32)
            xpad = load(bigp, d_xpad, [C, HP, WP], bf)
            ldiag = load(bigp, d_ldiag, [C, K2, C], bf)

            wkT = wpack[:, 0:C]
            wq = wpack[:, C:2 * C]
            wvfT = wpack[:, 2 * C:3 * C]
            wgT = wpack[:, 3 * C:4 * C]
            ident = wpack[:, 4 * C:5 * C]
            wwT = wpack[:K2, 5 * C:5 * C + K2]
            ones49 = wpack[:K2, 5 * C + K2:5 * C + K2 + 1]
            ones1_49 = wpack[:1, 5 * C + K2 + 1:5 * C + 2 * K2 + 1]
            bqcol = wpack[:, 5 * C + 2 * K2 + 1:5 * C + 2 * K2 + 2]

            bvfcol = fpack[:, 0:1]
            bkcol = fpack[:, 1:2]
            bfcol = fpack[:, 2:3]
            gbs = fpack[:, 3:4]
            gbb = fpack[:, 4:5]
            bwcol = fpack[:K2, 5:6]
            caw1T = fpack[:, 6:14]
            caw2T = fpack[:8, 14:14 + C]

            # persistent lepe accumulator: 4 chunks x 1 bank
            lp = lepep.tile([C, NCHUNK, 512], f32, tag="lp")

            # lepe diag matmul emitter (tap-major; interleaved as PE filler)
            lepe_iter = iter(range(K2))

            def emit_lepe(n_taps):
                for _ in range(n_taps):
                    k = next(lepe_iter, None)
                    if k is None:
                        return
                    kh, kw = divmod(k, K)
                    for ci in range(NCHUNK):
                        rhs = xpad[:, kh + ci * 7:kh + ci * 7 + 7, kw:kw + W]
                        nc.tensor.matmul(
                            lp[:, ci, :CH_N], lhsT=ldiag[:, k, :], rhs=rhs,
                            start=(k == 0), stop=False,
                        )

            # ============ phase A ============
            with (
                tc.tile_pool(name="psA", bufs=1, space="PSUM") as psA,
                tc.tile_pool(name="mmA", bufs=2, space="PSUM") as mmA,
            ):
                # pooled key: x_up [C,56,56] -> block sum 8x8 -> [C,7,7]
                pool1 = bigp.tile([C, H, 7], f32)
                nc.vector.tensor_reduce(
                    out=pool1[:],
                    in_=xup[:].rearrange("p h (bw dw) -> p h bw dw", dw=8),
                    axis=AX.X, op=ALU.add,
                )
                pooledf = workp.tile([C, 7, 7], f32, tag="pooledf")
                nc.vector.tensor_reduce(
                    out=pooledf[:],
                    in_=pool1[:].rearrange("p (bh dh) bw -> p bh bw dh", dh=8),
                    axis=AX.X, op=ALU.add,
                )
                pooled = workp.tile([C, K2], bf, tag="pooled")
                nc.vector.tensor_copy(pooled[:], pooledf[:].rearrange("p a b -> p (a b)"))

                # key = wk @ pooled + bk   (wk pre-divided by 64 on host)
                key_ps = mmA.tile([C, 512], f32, tag="mm")
                nc.tensor.matmul(key_ps[:, :K2], lhsT=wkT, rhs=pooled[:], start=True, stop=True)
                key = workp.tile([C, K2], bf, tag="key")
                nc.scalar.activation(key[:], key_ps[:, :K2], AF.Identity, bias=bkcol, scale=1.0)

                # eb = SCALE * key^T bq  -> [49,1] exp bias
                eb_ps = mmA.tile([K2, 512], f32, tag="mm")
                nc.tensor.matmul(eb_ps[:, :1], lhsT=key[:], rhs=bqcol, start=True, stop=True)
                eb = constp.tile([K2, 1], f32)
                nc.scalar.mul(eb[:], eb_ps[:, :1], SCALE)

                # KW = wq^T @ key  [C, 49]
                kw_ps = mmA.tile([C, 512], f32, tag="mm")
                nc.tensor.matmul(kw_ps[:, :K2], lhsT=wq, rhs=key[:], start=True, stop=True)
                KW = workp.tile([C, K2], bf, tag="KW")
                nc.scalar.copy(KW[:], kw_ps[:, :K2])

                # scores -> exp  (scores = KW^T @ x_skip) -- the dynw critical
                # chain runs lepe-free so the tap loop can start ASAP
                expv = bigp.tile([K2, N], bf)
                for hi in range(2):
                    sc_ps = psA.tile([K2, 2, 512], f32, tag="big49")
                    for cj in range(2):
                        ci = hi * 2 + cj
                        rhs = xpad[:, 3 + ci * 7:3 + ci * 7 + 7, 3:3 + W]
                        nc.tensor.matmul(sc_ps[:, cj, :CH_N], lhsT=KW[:], rhs=rhs, start=True, stop=True)
                    nc.scalar.activation(
                        expv[:, hi * HN:(hi + 1) * HN].rearrange("p (a x) -> p a x", a=2),
                        sc_ps[:, :, :CH_N], AF.Exp, bias=eb[:], scale=SCALE,
                    )

                # Z row sums -> 1/Z (fast approx reciprocal on the idle
                # prefix DVE; bw joins dynw additively after normalization)
                zsb = workp.tile([1, N], f32, tag="zsb")
                for ci in range(NCHUNK):
                    z_ps = mmA.tile([1, 512], f32, tag="mm")
                    sl = slice(ci * CH_N, (ci + 1) * CH_N)
                    nc.tensor.matmul(z_ps[:, :CH_N], lhsT=ones49, rhs=expv[:, sl], start=True, stop=True)
                    nc.scalar.copy(zsb[:, sl], z_ps[:, :CH_N])
                rec_rowf = workp.tile([1, N], f32, tag="rec_rowf")
                nc.vector.reciprocal_approx_fast(rec_rowf[:], zsb[:])
                rec_row = workp.tile([1, N], bf, tag="rec_row")
                nc.vector.tensor_copy(rec_row[:], rec_rowf[:])

                # recbc = broadcast 1/Z to 49 partitions; dynw = (wwT@exp + bw x Z) * recbc
                dynw = bigp.tile([K2, N], bf)
                recbc = bigp.tile([K2, N], bf)
                for hi in range(2):
                    rb_ps = psA.tile([K2, 2, 512], f32, tag="big49")
                    for cj in range(2):
                        ci = hi * 2 + cj
                        sl = slice(ci * CH_N, (ci + 1) * CH_N)
                        nc.tensor.matmul(rb_ps[:, cj, :CH_N], lhsT=ones1_49, rhs=rec_row[:, sl], start=True, stop=True)
                    nc.scalar.copy(
                        recbc[:, hi * HN:(hi + 1) * HN].rearrange("p (a x) -> p a x", a=2),
                        rb_ps[:, :, :CH_N],
                    )
                for hi in range(2):
                    dU_ps = psA.tile([K2, 2, 512], f32, tag="big49")
                    for cj in range(2):
                        ci = hi * 2 + cj
                        sl = slice(ci * CH_N, (ci + 1) * CH_N)
                        nc.tensor.matmul(dU_ps[:, cj, :CH_N], lhsT=wwT, rhs=expv[:, sl], start=True, stop=True)
                    hsl = slice(hi * HN, (hi + 1) * HN)
                    nc.vector.tensor_mul(
                        dynw[:, hsl].rearrange("p (a x) -> p a x", a=2),
                        dU_ps[:, :, :CH_N],
                        recbc[:, hsl].rearrange("p (a x) -> p a x", a=2),
                    )
                # dyn_w = ww@attn + bw  (bw joins after the 1/Z normalization)
                nc.vector.tensor_scalar_add(dynw[:], dynw[:], bwcol)

                # ship dynw to DRAM for the per-tap replicate DMAs
                nc.sync.dma_start(out=d_dynw[:], in_=dynw[:])

                # V' = (fus_w @ wv) @ x_skip, zero-padded borders
                vpad = bigp.tile([C, HP, WP], bf)
                nc.vector.memset(vpad[:], 0.0)
                for ci in range(5):  # 34 rows: 7,7,7,7,6
                    r = ci * 7
                    nr = min(7, HP - r)
                    v_ps = mmA.tile([C, 512], f32, tag="mm")
                    rhs = xpad[:, r:r + nr, 3:3 + W]
                    nc.tensor.matmul(v_ps[:, :nr * W], lhsT=wvfT, rhs=rhs, start=True, stop=True)
                    nc.scalar.activation(
                        vpad[:, r:r + nr, 3:3 + W], v_ps[:, :nr * W], AF.Identity,
                        bias=bvfcol, scale=1.0,
                    )
                    emit_lepe(1)

                # gate = silu(bn(wg @ x_skip)) in bf16
                gate = bigp.tile([C, N], bf)
                for ci in range(NCHUNK):
                    g_ps = mmA.tile([C, 512], f32, tag="mm")
                    rhs = xpad[:, 3 + ci * 7:3 + ci * 7 + 7, 3:3 + W]
                    nc.tensor.matmul(g_ps[:, :CH_N], lhsT=wgT, rhs=rhs, start=True, stop=True)
                    nc.scalar.activation(
                        gate[:, ci * CH_N:(ci + 1) * CH_N], g_ps[:, :CH_N], AF.Silu,
                        bias=gbb, scale=gbs,
                    )
                    emit_lepe(1)

            # ============ phase B: 49-tap dynamic conv + lepe filler ============
            # Odd taps >=5 send their product to the (otherwise idle) PE,
            # which accumulates them into the lepe PSUM via identity matmuls;
            # even taps accumulate on the DVE in bf16 chains.
            PE_ADD = frozenset({45, 47})
            acc = [bigp.tile([C, N], bf, name=f"acc{j}", tag=f"acc{j}") for j in range(4)]
            dve_first = [True] * 4
            dma_engines = [nc.sync, nc.scalar]
            TG = 4  # taps per replicate DMA (amortizes per-packet overhead)

            def pe_accumulate(tmp, stop):
                for ci in range(NCHUNK):
                    nc.tensor.matmul(
                        lp[:, ci, :CH_N], lhsT=ident,
                        rhs=tmp[:, ci * 7:(ci + 1) * 7, :],
                        start=False, stop=stop,
                    )

            tmp47 = None
            bounds = [0, 1, 3]
            while bounds[-1] < K2:
                bounds.append(min(bounds[-1] + TG, K2))
            tap_groups = [list(range(a, b)) for a, b in zip(bounds, bounds[1:])]
            for gi, grp in enumerate(tap_groups):
                # broadcast dynw rows grp to all 128 partitions via stride-0
                # DMA, alternating between the two HWDGE queues
                bcb = pipep.tile([C, TG, HALF, W], bf, tag="bcb", bufs=3)
                nt = len(grp)
                dma_engines[(gi + 1) % 2].dma_start(
                    out=bcb[:, :nt, :, :],
                    in_=d_dynw[grp[0]:grp[0] + nt, :].rearrange(
                        "t (r w) -> t r w", w=W).partition_broadcast(C))
                for k in grp:
                    kh, kw = divmod(k, K)
                    vap = vpad[:, kh:kh + HALF, kw:kw + W]
                    bck = bcb[:, k - grp[0], :, :]
                    if k in PE_ADD:
                        if k == 47:
                            tmp47 = pipep.tile([C, HALF, W], bf, tag="tmp47", bufs=1)
                            nc.vector.tensor_mul(tmp47[:], vap, bck)
                        else:
                            tmp = pipep.tile([C, HALF, W], bf, tag="ptmp", bufs=3)
                            nc.vector.tensor_mul(tmp[:], vap, bck)
                            pe_accumulate(tmp[:], stop=False)
                    else:
                        j = k % 4
                        tgt = acc[j]
                        first = dve_first[j]
                        dve_first[j] = False
                        osl = tgt[:].rearrange("p (r w) -> p r w", w=W)
                        if first:
                            nc.vector.tensor_mul(osl, vap, bck)
                        else:
                            tmp = pipep.tile([C, HALF, W], bf, tag="dtmp")
                            nc.vector.tensor_mul(tmp[:], vap, bck)
                            nc.vector.tensor_add(osl, osl, tmp[:])
                    emit_lepe(1)

            emit_lepe(K2)  # drain any remaining lepe taps
            # tap 47 closes the lepe/PE-add accumulation group
            pe_accumulate(tmp47[:], stop=True)

            # ============ tail ============
            xid = load(bigp, d_xid, [C, N], f32)
            a01 = bigp.tile([C, N], bf)
            nc.vector.tensor_add(a01[:], acc[0][:], acc[1][:])
            a23 = bigp.tile([C, N], bf)
            nc.vector.tensor_add(a23[:], acc[2][:], acc[3][:])
            pre = bigp.tile([C, N], bf)
            nc.vector.tensor_add(pre[:], a01[:], a23[:])

            xf = bigp.tile([C, N], bf)
            scol = workp.tile([C, NCHUNK], f32, tag="scol")
            for ci in range(NCHUNK):
                sl = slice(ci * CH_N, (ci + 1) * CH_N)
                nc.vector.scalar_tensor_tensor(
                    out=xf[:, sl], in0=pre[:, sl], scalar=bfcol, in1=lp[:, ci, :CH_N],
                    op0=ALU.add, op1=ALU.add, accum_out=scol[:, ci:ci + 1],
                )

            g2 = bigp.tile([C, N], bf)
            nc.vector.tensor_mul(g2[:], gate[:], xf[:])

            # channel stats
            mx = workp.tile([C, 1], f32, tag="mx")
            nc.vector.tensor_reduce(out=mx[:], in_=xf[:], axis=AX.X, op=ALU.max)
            s01 = workp.tile([C, 1], f32, tag="s01")
            nc.vector.tensor_add(s01[:], scol[:, 0:1], scol[:, 1:2])
            s23 = workp.tile([C, 1], f32, tag="s23")
            nc.vector.tensor_add(s23[:], scol[:, 2:3], scol[:, 3:4])
            stats = workp.tile([C, 2], f32, tag="stats")
            nc.vector.tensor_add(stats[:, 0:1], s01[:], s23[:])
            nc.vector.tensor_copy(stats[:, 1:2], mx[:])

            stat4 = workp.tile([C, 4], f32, tag="stat4")
            if use_collectives:
                nc.sync.dma_start(out=cc_in[:], in_=stats[:])
                nc.gpsimd.collective_compute(
                    "AllGather", ALU.bypass, replica_groups=groups,
                    ins=[cc_in[:]], outs=[cc_out[:]],
                )
                nc.sync.dma_start(
                    out=stat4[:].rearrange("c (r s) -> c r s", r=2),
                    in_=cc_out[:].rearrange("(r c) s -> c r s", r=2))
            else:
                nc.vector.tensor_copy(stat4[:, 0:2], stats[:])
                nc.vector.tensor_copy(stat4[:, 2:4], stats[:])

            ca_in = workp.tile([C, 2], f32, tag="ca_in")
            nc.vector.tensor_add(ca_in[:, 0:1], stat4[:, 0:1], stat4[:, 2:3])
            nc.vector.tensor_scalar_mul(ca_in[:, 0:1], ca_in[:, 0:1], 1.0 / (H * W))
            nc.vector.tensor_tensor(
                out=ca_in[:, 1:2], in0=stat4[:, 1:2], in1=stat4[:, 3:4], op=ALU.max)

            with tc.tile_pool(name="mmC", bufs=2, space="PSUM") as mmC:
                r1_ps = mmC.tile([8, 512], f32, tag="mm")
                nc.tensor.matmul(r1_ps[:, :2], lhsT=caw1T, rhs=ca_in[:], start=True, stop=True)
                r1 = workp.tile([8, 2], f32, tag="r1")
                nc.scalar.activation(r1[:], r1_ps[:, :2], AF.Relu, bias=0.0, scale=1.0)
                r2_ps = mmC.tile([C, 512], f32, tag="mm")
                nc.tensor.matmul(r2_ps[:, :2], lhsT=caw2T, rhs=r1[:], start=True, stop=True)
                r2sb = workp.tile([C, 2], f32, tag="r2sb")
                nc.vector.tensor_copy(r2sb[:], r2_ps[:, :2])
                casum = workp.tile([C, 1], f32, tag="casum")
                nc.vector.tensor_add(casum[:], r2sb[:, 0:1], r2sb[:, 1:2])
                ca = workp.tile([C, 1], f32, tag="ca")
                nc.scalar.activation(ca[:], casum[:], AF.Sigmoid, bias=0.0, scale=1.0)

            outt = bigp.tile([C, N], f32)
            nc.vector.scalar_tensor_tensor(
                out=outt[:], in0=g2[:], scalar=ca[:], in1=xid[:],
                op0=ALU.mult, op1=ALU.add,
            )
            nc.sync.dma_start(out=d_y[:], in_=outt[:])

    nc.compile()
    return nc


def _host_prep(inputs):
    """Build per-core input maps (slicing / dtype casts / small weight folds)."""
    f = {k: np.asarray(v, dtype=np.float32) for k, v in inputs.items()}

    # lepe reparam: fold 5 depthwise convs + BNs into one 7x7 kernel + bias
    w7 = f["lk_w"][:, 0] * f["lk_bn_s"][:, None, None]
    w7[:, 1:6, 1:6] += f["dw5"][:, 0] * f["bn5_s"][:, None, None]
    w7[:, 2:5, 2:5] += f["dw3a"][:, 0] * f["bn3a_s"][:, None, None]
    w7[:, 1::2, 1::2] += f["dw3b"][:, 0] * f["bn3b_s"][:, None, None]
    w7[:, ::3, ::3] += f["dw3c"][:, 0] * f["bn3c_s"][:, None, None]
    W_eff = f["lepe_bn_s"][:, None, None] * w7
    b_eff = (
        f["lepe_bn_s"]
        * (f["lk_bn_b"] + f["bn5_b"] + f["bn3a_b"] + f["bn3b_b"] + f["bn3c_b"])
        + f["lepe_bn_b"]
    )
    bias_fused = (b_eff + f["fus_b"]).astype(np.float32)

    ldiag = np.zeros((C, K2, C), dtype=BF16)
    cc = np.arange(C)
    for k in range(K2):
        ldiag[cc, k, cc] = W_eff[:, k // K, k % K].astype(BF16)

    wvf = f["fus_w"] @ f["wv"]           # folded value conv
    bvf = f["fus_w"] @ f["bv"]           # its bias (applied inside vpad interior)

    # bf16 pack: wkT | wq | wvfT | wgT | ident | wwT | ones49 | ones1_49 | bq
    wpack = np.zeros((C, 5 * C + 2 * K2 + 2), dtype=BF16)
    wpack[:, 0:C] = (f["wk"] / 64.0).T.astype(BF16)
    wpack[:, C:2 * C] = f["wq"].astype(BF16)
    wpack[:, 2 * C:3 * C] = wvf.T.astype(BF16)
    wpack[:, 3 * C:4 * C] = f["gate_w"].T.astype(BF16)
    wpack[:, 4 * C:5 * C] = np.eye(C, dtype=BF16)
    wpack[:K2, 5 * C:5 * C + K2] = f["ww"].T.astype(BF16)
    wpack[:K2, 5 * C + K2] = 1.0
    wpack[0, 5 * C + K2 + 1:5 * C + 2 * K2 + 1] = 1.0
    wpack[:, 5 * C + 2 * K2 + 1] = f["bq"].astype(BF16)

    # f32 pack: bvf | bk | biasf | gbs | gbb | bw | caw1T | caw2T
    fpack = np.zeros((C, 6 + 8 + C), dtype=np.float32)
    fpack[:, 0] = bvf
    fpack[:, 1] = f["bk"]
    fpack[:, 2] = bias_fused
    fpack[:, 3] = f["gate_bn_s"]
    fpack[:, 4] = f["gate_bn_b"]
    fpack[:K2, 5] = f["bw"]
    fpack[:, 6:14] = f["ca_w1"].T
    fpack[:8, 14:14 + C] = f["ca_w2"].T

    common = {
        "lepe_diag": ldiag,
        "wpack": wpack,
        "fpack": fpack,
    }

    xsk_pad = np.zeros((B, C, H + 6, W + 6), dtype=np.float32)
    xsk_pad[:, :, 3:3 + H, 3:3 + W] = f["x_skip"]
    rs = float(f["res_scale"][0])

    in_maps = []
    for core in range(8):
        b, half = divmod(core, 2)
        r0 = half * HALF
        m = dict(common)
        m["x_pad"] = xsk_pad[b, :, r0:r0 + HP, :].astype(BF16)
        m["x_id"] = (f["x_skip"][b, :, r0:r0 + HALF, :].reshape(C, N) * rs).astype(np.float32)
        m["x_up"] = f["x_up"][b].astype(BF16)
        in_maps.append(m)
    return in_maps


def kernel(**inputs):
    if "nc" not in _CACHE:
        _CACHE["nc"] = _build_program()
    nc = _CACHE["nc"]
    in_maps = _host_prep(inputs)
    res = run_bass_kernel_spmd(nc, in_maps, list(range(8)))
    out = np.empty((B, C, H, W), dtype=np.float32)
    for core in range(8):
        b, half = divmod(core, 2)
        r0 = half * HALF
        out[b, :, r0:r0 + HALF, :] = res.results[core]["y"].reshape(C, HALF, W)
    return out


# revision 38
# speedup vs baseline: 1.0813x; 1.0813x over previous
"""Trainium2 Bass kernel for nn_CGSC_64914135712264.

Sharding: 8 cores = (batch b in 0..3) x (H-half in 0..1). Each core computes
a [C=128, 28, 56] output slab of its batch. All params replicated.

v2 design (restructured from v1 for engine balance):
  - fus_w folded into the value conv (V' = (fus_w@wv)@x_skip); the bias term
    rides a rank-1 accumulating matmul with SD = sum_k dynw[k].  The 49-tap
    dynamic conv then produces x_fused directly.
  - q never materialized: scores = (wq^T key)^T @ x_skip (KW folding).
  - Per tap: PE broadcasts dynw row (PSUM) -> ACT casts to bf16 SBUF ->
    DVE does bf16 2x mul + add into one of 4 accumulator chains.  A subset
    of taps runs mul+add on GPSIMD into its own chain.
  - lepe diag-matmuls are interleaved into the PE stream as filler behind
    the broadcasts so the PE stays dense and HAM-warm.
  - 1/Z via ACT exp(-ln(Z)) instead of the 8-pass DVE reciprocal.
  - Channel attention: one AllGather (pair groups) instead of two serial
    AllReduces, with a dummy warm-up collective issued at kernel start to
    absorb the ~30us cold-start of the CC path.
"""

import sys

sys.path.insert(0, "/opt/trn_rl_repo")

import numpy as np
import ml_dtypes

import concourse.bass as bass
import concourse.mybir as mybir
import concourse.tile as tile
from concourse import bacc
from concourse.bass_utils import run_bass_kernel_spmd

BF16 = ml_dtypes.bfloat16
F32 = mybir.dt.float32
BF = mybir.dt.bfloat16

B, C, H, W = 4, 128, 56, 56
K = 7
K2 = 49
HALF = 28          # rows per core
HP, WP = HALF + 6, W + 6   # padded tile 34 x 62
N = HALF * W       # 1568 free elems per core
NCHUNK = 4
ROWS_PER_CHUNK = 7
CH_N = ROWS_PER_CHUNK * W  # 392
HN = 2 * CH_N              # 784 (half-tap free size, 14 rows)
SCALE = float(C) ** -0.5

# GPSIMD shares the DVE SBUF port (exclusive lock) -- offloading elementwise
# taps to it stalls the DVE, so everything elementwise stays on the DVE.
GP_TAPS = frozenset()

_CACHE = {}


def _build_program(use_collectives=True):
    nc = bacc.Bacc("TRN2", target_bir_lowering=False, debug=False, num_devices=8)
    f32, bf = F32, BF

    # ---- DRAM I/O (declaration order ~= DMA priority) ----
    d_xup = nc.dram_tensor("x_up", [C, H, W], bf, kind="ExternalInput")
    d_xpad = nc.dram_tensor("x_pad", [C, HP, WP], bf, kind="ExternalInput")
    # bf16 pack: wkT | wq | wvfT | wgT | ident | wwT(pad128) | ones49 | ones1_49 | bq
    d_wpack = nc.dram_tensor("wpack", [C, 5 * C + 2 * K2 + 2], bf, kind="ExternalInput")
    # f32 pack: bvf | bk | biasf | gbs | gbb | bw(pad128) | caw1T | caw2T(pad128)
    d_fpack = nc.dram_tensor("fpack", [C, 6 + 8 + C], f32, kind="ExternalInput")
    d_ldiag = nc.dram_tensor("lepe_diag", [C, K2, C], bf, kind="ExternalInput")
    d_xid = nc.dram_tensor("x_id", [C, N], f32, kind="ExternalInput")
    d_y = nc.dram_tensor("y", [C, N], f32, kind="ExternalOutput")
    d_dynw = nc.dram_tensor("dynw_hbm", [K2, N], bf)

    # collectives scratch
    cc_warm_in = nc.dram_tensor("cc_warm_in", [1, 8], f32)
    cc_warm_out = nc.dram_tensor("cc_warm_out", [2, 8], f32)
    cc_in = nc.dram_tensor("cc_in", [C, 2], f32)
    cc_out = nc.dram_tensor("cc_out", [2 * C, 2], f32)
    groups = [[0, 1], [2, 3], [4, 5], [6, 7]]

    AF = mybir.ActivationFunctionType
    ALU = mybir.AluOpType
    AX = mybir.AxisListType

    with tile.TileContext(nc, trace_sim=False) as tc:
        with (
            tc.tile_pool(name="const", bufs=1) as constp,
            tc.tile_pool(name="big", bufs=1) as bigp,
            tc.tile_pool(name="work", bufs=2) as workp,
            tc.tile_pool(name="pipe", bufs=3) as pipep,
            tc.tile_pool(name="lepe_ps", bufs=1, space="PSUM") as lepep,
        ):
            def load(pool, dram, shape, dtype):
                t = pool.tile(shape, dtype, tag=dram.name)
                nc.sync.dma_start(out=t[:], in_=dram[:])
                return t

            # warm up the CC path early (result unused)
            if use_collectives:
                nc.gpsimd.collective_compute(
                    "AllGather", ALU.bypass, replica_groups=groups,
                    ins=[cc_warm_in[:]], outs=[cc_warm_out[:]],
                )

            xup = load(bigp, d_xup, [C, H, W], bf)
            wpack = load(constp, d_wpack, [C, 5 * C + 2 * K2 + 2], bf)
            fpack = load(constp, d_fpack, [C, 6 + 8 + C], f32)
            xpad = load(bigp, d_xpad, [C, HP, WP], bf)
            ldiag = load(bigp, d_ldiag, [C, K2, C], bf)

            wkT = wpack[:, 0:C]
            wq = wpack[:, C:2 * C]
            wvfT = wpack[:, 2 * C:3 * C]
            wgT = wpack[:, 3 * C:4 * C]
            ident = wpack[:, 4 * C:5 * C]
            wwT = wpack[:K2, 5 * C:5 * C + K2]
            ones49 = wpack[:K2, 5 * C + K2:5 * C + K2 + 1]
            ones1_49 = wpack[:1, 5 * C + K2 + 1:5 * C + 2 * K2 + 1]
            bqcol = wpack[:, 5 * C + 2 * K2 + 1:5 * C + 2 * K2 + 2]

            bvfcol = fpack[:, 0:1]
            bkcol = fpack[:, 1:2]
            bfcol = fpack[:, 2:3]
            gbs = fpack[:, 3:4]
            gbb = fpack[:, 4:5]
            bwcol = fpack[:K2, 5:6]
            caw1T = fpack[:, 6:14]
            caw2T = fpack[:8, 14:14 + C]

            # persistent lepe accumulator: 4 chunks x 1 bank
            lp = lepep.tile([C, NCHUNK, 512], f32, tag="lp")

            # lepe diag matmul emitter (tap-major; interleaved as PE filler)
            lepe_iter = iter(range(K2))

            def emit_lepe(n_taps):
                for _ in range(n_taps):
                    k = next(lepe_iter, None)
                    if k is None:
                        return
                    kh, kw = divmod(k, K)
                    for ci in range(NCHUNK):
                        rhs = xpad[:, kh + ci * 7:kh + ci * 7 + 7, kw:kw + W]
                        nc.tensor.matmul(
                            lp[:, ci, :CH_N], lhsT=ldiag[:, k, :], rhs=rhs,
                            start=(k == 0), stop=False,
                        )

            # ============ phase A ============
            with (
                tc.tile_pool(name="psA", bufs=1, space="PSUM") as psA,
                tc.tile_pool(name="mmA", bufs=2, space="PSUM") as mmA,
            ):
                # pooled key: x_up [C,56,56] -> block sum 8x8 -> [C,7,7]
                pool1 = bigp.tile([C, H, 7], f32)
                nc.vector.tensor_reduce(
                    out=pool1[:],
                    in_=xup[:].rearrange("p h (bw dw) -> p h bw dw", dw=8),
                    axis=AX.X, op=ALU.add,
                )
                pooledf = workp.tile([C, 7, 7], f32, tag="pooledf")
                nc.vector.tensor_reduce(
                    out=pooledf[:],
                    in_=pool1[:].rearrange("p (bh dh) bw -> p bh bw dh", dh=8),
                    axis=AX.X, op=ALU.add,
                )
                pooled = workp.tile([C, K2], bf, tag="pooled")
                nc.vector.tensor_copy(pooled[:], pooledf[:].rearrange("p a b -> p (a b)"))

                # key = wk @ pooled + bk   (wk pre-divided by 64 on host)
                key_ps = mmA.tile([C, 512], f32, tag="mm")
                nc.tensor.matmul(key_ps[:, :K2], lhsT=wkT, rhs=pooled[:], start=True, stop=True)
                key = workp.tile([C, K2], bf, tag="key")
                nc.scalar.activation(key[:], key_ps[:, :K2], AF.Identity, bias=bkcol, scale=1.0)

                # eb = SCALE * key^T bq  -> [49,1] exp bias
                eb_ps = mmA.tile([K2, 512], f32, tag="mm")
                nc.tensor.matmul(eb_ps[:, :1], lhsT=key[:], rhs=bqcol, start=True, stop=True)
                eb = constp.tile([K2, 1], f32)
                nc.scalar.mul(eb[:], eb_ps[:, :1], SCALE)

                # KW = wq^T @ key  [C, 49]
                kw_ps = mmA.tile([C, 512], f32, tag="mm")
                nc.tensor.matmul(kw_ps[:, :K2], lhsT=wq, rhs=key[:], start=True, stop=True)
                KW = workp.tile([C, K2], bf, tag="KW")
                nc.scalar.copy(KW[:], kw_ps[:, :K2])

                # scores -> exp  (scores = KW^T @ x_skip) -- the dynw critical
                # chain runs lepe-free so the tap loop can start ASAP
                expv = bigp.tile([K2, N], bf)
                for hi in range(2):
                    sc_ps = psA.tile([K2, 2, 512], f32, tag="big49")
                    for cj in range(2):
                        ci = hi * 2 + cj
                        rhs = xpad[:, 3 + ci * 7:3 + ci * 7 + 7, 3:3 + W]
                        nc.tensor.matmul(sc_ps[:, cj, :CH_N], lhsT=KW[:], rhs=rhs, start=True, stop=True)
                    nc.scalar.activation(
                        expv[:, hi * HN:(hi + 1) * HN].rearrange("p (a x) -> p a x", a=2),
                        sc_ps[:, :, :CH_N], AF.Exp, bias=eb[:], scale=SCALE,
                    )

                # Z row sums -> 1/Z (fast approx reciprocal on the idle
                # prefix DVE; bw joins dynw additively after normalization)
                zsb = workp.tile([1, N], f32, tag="zsb")
                for ci in range(NCHUNK):
                    z_ps = mmA.tile([1, 512], f32, tag="mm")
                    sl = slice(ci * CH_N, (ci + 1) * CH_N)
                    nc.tensor.matmul(z_ps[:, :CH_N], lhsT=ones49, rhs=expv[:, sl], start=True, stop=True)
                    nc.scalar.copy(zsb[:, sl], z_ps[:, :CH_N])
                rec_rowf = workp.tile([1, N], f32, tag="rec_rowf")
                nc.vector.reciprocal_approx_fast(rec_rowf[:], zsb[:])
                rec_row = workp.tile([1, N], bf, tag="rec_row")
                nc.vector.tensor_copy(rec_row[:], rec_rowf[:])

                # recbc = broadcast 1/Z to 49 partitions; dynw = (wwT@exp + bw x Z) * recbc
                dynw = bigp.tile([K2, N], bf)
                recbc = bigp.tile([K2, N], bf)
                for hi in range(2):
                    rb_ps = psA.tile([K2, 2, 512], f32, tag="big49")
                    for cj in range(2):
                        ci = hi * 2 + cj
                        sl = slice(ci * CH_N, (ci + 1) * CH_N)
                        nc.tensor.matmul(rb_ps[:, cj, :CH_N], lhsT=ones1_49, rhs=rec_row[:, sl], start=True, stop=True)
                    nc.scalar.copy(
                        recbc[:, hi * HN:(hi + 1) * HN].rearrange("p (a x) -> p a x", a=2),
                        rb_ps[:, :, :CH_N],
                    )
                for hi in range(2):
                    dU_ps = psA.tile([K2, 2, 512], f32, tag="big49")
                    for cj in range(2):
                        ci = hi * 2 + cj
                        sl = slice(ci * CH_N, (ci + 1) * CH_N)
                        nc.tensor.matmul(dU_ps[:, cj, :CH_N], lhsT=wwT, rhs=expv[:, sl], start=True, stop=True)
                    hsl = slice(hi * HN, (hi + 1) * HN)
                    nc.vector.tensor_mul(
                        dynw[:, hsl].rearrange("p (a x) -> p a x", a=2),
                        dU_ps[:, :, :CH_N],
                        recbc[:, hsl].rearrange("p (a x) -> p a x", a=2),
                    )
                # dyn_w = ww@attn + bw  (bw joins after the 1/Z normalization)
                nc.vector.tensor_scalar_add(dynw[:], dynw[:], bwcol)

                # ship dynw to DRAM for the per-tap replicate DMAs
                nc.sync.dma_start(out=d_dynw[:], in_=dynw[:])

                # V' = (fus_w @ wv) @ x_skip, zero-padded borders
                vpad = bigp.tile([C, HP, WP], bf)
                nc.vector.memset(vpad[:], 0.0)
                for ci in range(5):  # 34 rows: 7,7,7,7,6
                    r = ci * 7
                    nr = min(7, HP - r)
                    v_ps = mmA.tile([C, 512], f32, tag="mm")
                    rhs = xpad[:, r:r + nr, 3:3 + W]
                    nc.tensor.matmul(v_ps[:, :nr * W], lhsT=wvfT, rhs=rhs, start=True, stop=True)
                    nc.scalar.activation(
                        vpad[:, r:r + nr, 3:3 + W], v_ps[:, :nr * W], AF.Identity,
                        bias=bvfcol, scale=1.0,
                    )
                    emit_lepe(1)

                # gate = silu(bn(wg @ x_skip)) in bf16
                gate = bigp.tile([C, N], bf)
                for ci in range(NCHUNK):
                    g_ps = mmA.tile([C, 512], f32, tag="mm")
                    rhs = xpad[:, 3 + ci * 7:3 + ci * 7 + 7, 3:3 + W]
                    nc.tensor.matmul(g_ps[:, :CH_N], lhsT=wgT, rhs=rhs, start=True, stop=True)
                    nc.scalar.activation(
                        gate[:, ci * CH_N:(ci + 1) * CH_N], g_ps[:, :CH_N], AF.Silu,
                        bias=gbb, scale=gbs,
                    )
                    emit_lepe(1)

            # ============ phase B: 49-tap dynamic conv + lepe filler ============
            # Odd taps >=5 send their product to the (otherwise idle) PE,
            # which accumulates them into the lepe PSUM via identity matmuls;
            # even taps accumulate on the DVE in bf16 chains.
            PE_ADD = frozenset(k for k in range(5, K2) if k % 2 == 1)
            acc = [bigp.tile([C, N], bf, name=f"acc{j}", tag=f"acc{j}") for j in range(4)]
            dve_first = [True] * 4
            dma_engines = [nc.sync, nc.scalar]
            TG = 4  # taps per replicate DMA (amortizes per-packet overhead)

            def pe_accumulate(tmp, stop):
                for ci in range(NCHUNK):
                    nc.tensor.matmul(
                        lp[:, ci, :CH_N], lhsT=ident,
                        rhs=tmp[:, ci * 7:(ci + 1) * 7, :],
                        start=False, stop=stop,
                    )

            tmp47 = None
            bounds = [0, 1, 3]
            while bounds[-1] < K2:
                bounds.append(min(bounds[-1] + TG, K2))
            tap_groups = [list(range(a, b)) for a, b in zip(bounds, bounds[1:])]
            for gi, grp in enumerate(tap_groups):
                # broadcast dynw rows grp to all 128 partitions via stride-0
                # DMA, alternating between the two HWDGE queues
                bcb = pipep.tile([C, TG, HALF, W], bf, tag="bcb", bufs=3)
                nt = len(grp)
                dma_engines[(gi + 1) % 2].dma_start(
                    out=bcb[:, :nt, :, :],
                    in_=d_dynw[grp[0]:grp[0] + nt, :].rearrange(
                        "t (r w) -> t r w", w=W).partition_broadcast(C))
                for k in grp:
                    kh, kw = divmod(k, K)
                    vap = vpad[:, kh:kh + HALF, kw:kw + W]
                    bck = bcb[:, k - grp[0], :, :]
                    if k in PE_ADD:
                        if k == 47:
                            tmp47 = pipep.tile([C, HALF, W], bf, tag="tmp47", bufs=1)
                            nc.vector.tensor_mul(tmp47[:], vap, bck)
                        else:
                            tmp = pipep.tile([C, HALF, W], bf, tag="ptmp", bufs=3)
                            nc.vector.tensor_mul(tmp[:], vap, bck)
                            pe_accumulate(tmp[:], stop=False)
                    else:
                        j = k % 4
                        tgt = acc[j]
                        first = dve_first[j]
                        dve_first[j] = False
                        osl = tgt[:].rearrange("p (r w) -> p r w", w=W)
                        if first:
                            nc.vector.tensor_mul(osl, vap, bck)
                        else:
                            tmp = pipep.tile([C, HALF, W], bf, tag="dtmp")
                            nc.vector.tensor_mul(tmp[:], vap, bck)
                            nc.vector.tensor_add(osl, osl, tmp[:])
                    emit_lepe(1)

            emit_lepe(K2)  # drain any remaining lepe taps
            # tap 47 closes the lepe/PE-add accumulation group
            pe_accumulate(tmp47[:], stop=True)

            # ============ tail ============
            xid = load(bigp, d_xid, [C, N], f32)
            a01 = bigp.tile([C, N], bf)
            nc.vector.tensor_add(a01[:], acc[0][:], acc[1][:])
            a23 = bigp.tile([C, N], bf)
            nc.vector.tensor_add(a23[:], acc[2][:], acc[3][:])
            pre = bigp.tile([C, N], bf)
            nc.vector.tensor_add(pre[:], a01[:], a23[:])

            xf = bigp.tile([C, N], bf)
            scol = workp.tile([C, NCHUNK], f32, tag="scol")
            for ci in range(NCHUNK):
                sl = slice(ci * CH_N, (ci + 1) * CH_N)
                nc.vector.scalar_tensor_tensor(
                    out=xf[:, sl], in0=pre[:, sl], scalar=bfcol, in1=lp[:, ci, :CH_N],
                    op0=ALU.add, op1=ALU.add, accum_out=scol[:, ci:ci + 1],
                )

            g2 = bigp.tile([C, N], bf)
            nc.vector.tensor_mul(g2[:], gate[:], xf[:])

            # channel stats
            mx = workp.tile([C, 1], f32, tag="mx")
            nc.vector.tensor_reduce(out=mx[:], in_=xf[:], axis=AX.X, op=ALU.max)
            s01 = workp.tile([C, 1], f32, tag="s01")
            nc.vector.tensor_add(s01[:], scol[:, 0:1], scol[:, 1:2])
            s23 = workp.tile([C, 1], f32, tag="s23")
            nc.vector.tensor_add(s23[:], scol[:, 2:3], scol[:, 3:4])
            stats = workp.tile([C, 2], f32, tag="stats")
            nc.vector.tensor_add(stats[:, 0:1], s01[:], s23[:])
            nc.vector.tensor_copy(stats[:, 1:2], mx[:])

            stat4 = workp.tile([C, 4], f32, tag="stat4")
            if use_collectives:
                nc.sync.dma_start(out=cc_in[:], in_=stats[:])
                nc.gpsimd.collective_compute(
                    "AllGather", ALU.bypass, replica_groups=groups,
                    ins=[cc_in[:]], outs=[cc_out[:]],
                )
                nc.sync.dma_start(
                    out=stat4[:].rearrange("c (r s) -> c r s", r=2),
                    in_=cc_out[:].rearrange("(r c) s -> c r s", r=2))
            else:
                nc.vector.tensor_copy(stat4[:, 0:2], stats[:])
                nc.vector.tensor_copy(stat4[:, 2:4], stats[:])

            ca_in = workp.tile([C, 2], f32, tag="ca_in")
            nc.vector.tensor_add(ca_in[:, 0:1], stat4[:, 0:1], stat4[:, 2:3])
            nc.vector.tensor_scalar_mul(ca_in[:, 0:1], ca_in[:, 0:1], 1.0 / (H * W))
            nc.vector.tensor_tensor(
                out=ca_in[:, 1:2], in0=stat4[:, 1:2], in1=stat4[:, 3:4], op=ALU.max)

            with tc.tile_pool(name="mmC", bufs=2, space="PSUM") as mmC:
                r1_ps = mmC.tile([8, 512], f32, tag="mm")
                nc.tensor.matmul(r1_ps[:, :2], lhsT=caw1T, rhs=ca_in[:], start=True, stop=True)
                r1 = workp.tile([8, 2], f32, tag="r1")
                nc.scalar.activation(r1[:], r1_ps[:, :2], AF.Relu, bias=0.0, scale=1.0)
                r2_ps = mmC.tile([C, 512], f32, tag="mm")
                nc.tensor.matmul(r2_ps[:, :2], lhsT=caw2T, rhs=r1[:], start=True, stop=True)
                r2sb = workp.tile([C, 2], f32, tag="r2sb")
                nc.vector.tensor_copy(r2sb[:], r2_ps[:, :2])
                casum = workp.tile([C, 1], f32, tag="casum")
                nc.vector.tensor_add(casum[:], r2sb[:, 0:1], r2sb[:, 1:2])
                ca = workp.tile([C, 1], f32, tag="ca")
                nc.scalar.activation(ca[:], casum[:], AF.Sigmoid, bias=0.0, scale=1.0)

            outt = bigp.tile([C, N], f32)
            nc.vector.scalar_tensor_tensor(
                out=outt[:], in0=g2[:], scalar=ca[:], in1=xid[:],
                op0=ALU.mult, op1=ALU.add,
            )
            nc.sync.dma_start(out=d_y[:], in_=outt[:])

    nc.compile()
    return nc


def _host_prep(inputs):
    """Build per-core input maps (slicing / dtype casts / small weight folds)."""
    f = {k: np.asarray(v, dtype=np.float32) for k, v in inputs.items()}

    # lepe reparam: fold 5 depthwise convs + BNs into one 7x7 kernel + bias
    w7 = f["lk_w"][:, 0] * f["lk_bn_s"][:, None, None]
    w7[:, 1:6, 1:6] += f["dw5"][:, 0] * f["bn5_s"][:, None, None]
    w7[:, 2:5, 2:5] += f["dw3a"][:, 0] * f["bn3a_s"][:, None, None]
    w7[:, 1::2, 1::2] += f["dw3b"][:, 0] * f["bn3b_s"][:, None, None]
    w7[:, ::3, ::3] += f["dw3c"][:, 0] * f["bn3c_s"][:, None, None]
    W_eff = f["lepe_bn_s"][:, None, None] * w7
    b_eff = (
        f["lepe_bn_s"]
        * (f["lk_bn_b"] + f["bn5_b"] + f["bn3a_b"] + f["bn3b_b"] + f["bn3c_b"])
        + f["lepe_bn_b"]
    )
    bias_fused = (b_eff + f["fus_b"]).astype(np.float32)

    ldiag = np.zeros((C, K2, C), dtype=BF16)
    cc = np.arange(C)
    for k in range(K2):
        ldiag[cc, k, cc] = W_eff[:, k // K, k % K].astype(BF16)

    wvf = f["fus_w"] @ f["wv"]           # folded value conv
    bvf = f["fus_w"] @ f["bv"]           # its bias (applied inside vpad interior)

    # bf16 pack: wkT | wq | wvfT | wgT | ident | wwT | ones49 | ones1_49 | bq
    wpack = np.zeros((C, 5 * C + 2 * K2 + 2), dtype=BF16)
    wpack[:, 0:C] = (f["wk"] / 64.0).T.astype(BF16)
    wpack[:, C:2 * C] = f["wq"].astype(BF16)
    wpack[:, 2 * C:3 * C] = wvf.T.astype(BF16)
    wpack[:, 3 * C:4 * C] = f["gate_w"].T.astype(BF16)
    wpack[:, 4 * C:5 * C] = np.eye(C, dtype=BF16)
    wpack[:K2, 5 * C:5 * C + K2] = f["ww"].T.astype(BF16)
    wpack[:K2, 5 * C + K2] = 1.0
    wpack[0, 5 * C + K2 + 1:5 * C + 2 * K2 + 1] = 1.0
    wpack[:, 5 * C + 2 * K2 + 1] = f["bq"].astype(BF16)

    # f32 pack: bvf | bk | biasf | gbs | gbb | bw | caw1T | caw2T
    fpack = np.zeros((C, 6 + 8 + C), dtype=np.float32)
    fpack[:, 0] = bvf
    fpack[:, 1] = f["bk"]
    fpack[:, 2] = bias_fused
    fpack[:, 3] = f["gate_bn_s"]
    fpack[:, 4] = f["gate_bn_b"]
    fpack[:K2, 5] = f["bw"]
    fpack[:, 6:14] = f["ca_w1"].T
    fpack[:8, 14:14 + C] = f["ca_w2"].T

    common = {
        "lepe_diag": ldiag,
        "wpack": wpack,
        "fpack": fpack,
    }

    xsk_pad = np.zeros((B, C, H + 6, W + 6), dtype=np.float32)
    xsk_pad[:, :, 3:3 + H, 3:3 + W] = f["x_skip"]
    rs = float(f["res_scale"][0])

    in_maps = []
    for core in range(8):
        b, half = divmod(core, 2)
        r0 = half * HALF
        m = dict(common)
        m["x_pad"] = xsk_pad[b, :, r0:r0 + HP, :].astype(BF16)
        m["x_id"] = (f["x_skip"][b, :, r0:r0 + HALF, :].reshape(C, N) * rs).astype(np.float32)
        m["x_up"] = f["x_up"][b].astype(BF16)
        in_maps.append(m)
    return in_maps


def kernel(**inputs):
    if "nc" not in _CACHE:
        _CACHE["nc"] = _build_program()
    nc = _CACHE["nc"]
    in_maps = _host_prep(inputs)
    res = run_bass_kernel_spmd(nc, in_maps, list(range(8)))
    out = np.empty((B, C, H, W), dtype=np.float32)
    for core in range(8):
        b, half = divmod(core, 2)
        r0 = half * HALF
        out[b, :, r0:r0 + HALF, :] = res.results[core]["y"].reshape(C, HALF, W)
    return out


# revision 39
# speedup vs baseline: 1.0946x; 1.0123x over previous
"""Trainium2 Bass kernel for nn_CGSC_64914135712264.

Sharding: 8 cores = (batch b in 0..3) x (H-half in 0..1). Each core computes
a [C=128, 28, 56] output slab of its batch. All params replicated.

v2 design (restructured from v1 for engine balance):
  - fus_w folded into the value conv (V' = (fus_w@wv)@x_skip); the bias term
    rides a rank-1 accumulating matmul with SD = sum_k dynw[k].  The 49-tap
    dynamic conv then produces x_fused directly.
  - q never materialized: scores = (wq^T key)^T @ x_skip (KW folding).
  - Per tap: PE broadcasts dynw row (PSUM) -> ACT casts to bf16 SBUF ->
    DVE does bf16 2x mul + add into one of 4 accumulator chains.  A subset
    of taps runs mul+add on GPSIMD into its own chain.
  - lepe diag-matmuls are interleaved into the PE stream as filler behind
    the broadcasts so the PE stays dense and HAM-warm.
  - 1/Z via ACT exp(-ln(Z)) instead of the 8-pass DVE reciprocal.
  - Channel attention: one AllGather (pair groups) instead of two serial
    AllReduces, with a dummy warm-up collective issued at kernel start to
    absorb the ~30us cold-start of the CC path.
"""

import sys

sys.path.insert(0, "/opt/trn_rl_repo")

import numpy as np
import ml_dtypes

import concourse.bass as bass
import concourse.mybir as mybir
import concourse.tile as tile
from concourse import bacc
from concourse.bass_utils import run_bass_kernel_spmd

BF16 = ml_dtypes.bfloat16
F32 = mybir.dt.float32
BF = mybir.dt.bfloat16

B, C, H, W = 4, 128, 56, 56
K = 7
K2 = 49
HALF = 28          # rows per core
HP, WP = HALF + 6, W + 6   # padded tile 34 x 62
N = HALF * W       # 1568 free elems per core
NCHUNK = 4
ROWS_PER_CHUNK = 7
CH_N = ROWS_PER_CHUNK * W  # 392
HN = 2 * CH_N              # 784 (half-tap free size, 14 rows)
SCALE = float(C) ** -0.5

# GPSIMD shares the DVE SBUF port (exclusive lock) -- offloading elementwise
# taps to it stalls the DVE, so everything elementwise stays on the DVE.
GP_TAPS = frozenset()

_CACHE = {}


def _build_program(use_collectives=True):
    nc = bacc.Bacc("TRN2", target_bir_lowering=False, debug=False, num_devices=8)
    f32, bf = F32, BF

    # ---- DRAM I/O (declaration order ~= DMA priority) ----
    d_xup = nc.dram_tensor("x_up", [C, H, W], bf, kind="ExternalInput")
    d_xpad = nc.dram_tensor("x_pad", [C, HP, WP], bf, kind="ExternalInput")
    # bf16 pack: wkT | wq | wvfT | wgT | ident | wwT(pad128) | ones49 | ones1_49 | bq
    d_wpack = nc.dram_tensor("wpack", [C, 5 * C + 2 * K2 + 2], bf, kind="ExternalInput")
    # f32 pack: bvf | bk | biasf | gbs | gbb | bw(pad128) | caw1T | caw2T(pad128)
    d_fpack = nc.dram_tensor("fpack", [C, 6 + 8 + C], f32, kind="ExternalInput")
    d_ldiag = nc.dram_tensor("lepe_diag", [C, K2, C], bf, kind="ExternalInput")
    d_xid = nc.dram_tensor("x_id", [C, N], f32, kind="ExternalInput")
    d_y = nc.dram_tensor("y", [C, N], f32, kind="ExternalOutput")
    d_dynw = nc.dram_tensor("dynw_hbm", [K2, N], bf)

    # collectives scratch
    cc_warm_in = nc.dram_tensor("cc_warm_in", [1, 8], f32)
    cc_warm_out = nc.dram_tensor("cc_warm_out", [2, 8], f32)
    cc_in = nc.dram_tensor("cc_in", [C, 2], f32)
    cc_out = nc.dram_tensor("cc_out", [2 * C, 2], f32)
    groups = [[0, 1], [2, 3], [4, 5], [6, 7]]

    AF = mybir.ActivationFunctionType
    ALU = mybir.AluOpType
    AX = mybir.AxisListType

    with tile.TileContext(nc, trace_sim=False) as tc:
        with (
            tc.tile_pool(name="const", bufs=1) as constp,
            tc.tile_pool(name="big", bufs=1) as bigp,
            tc.tile_pool(name="work", bufs=2) as workp,
            tc.tile_pool(name="pipe", bufs=3) as pipep,
            tc.tile_pool(name="lepe_ps", bufs=1, space="PSUM") as lepep,
        ):
            def load(pool, dram, shape, dtype):
                t = pool.tile(shape, dtype, tag=dram.name)
                nc.sync.dma_start(out=t[:], in_=dram[:])
                return t

            # warm up the CC path early (result unused)
            if use_collectives:
                nc.gpsimd.collective_compute(
                    "AllGather", ALU.bypass, replica_groups=groups,
                    ins=[cc_warm_in[:]], outs=[cc_warm_out[:]],
                )

            xup = load(bigp, d_xup, [C, H, W], bf)
            wpack = load(constp, d_wpack, [C, 5 * C + 2 * K2 + 2], bf)
            fpack = load(constp, d_fpack, [C, 6 + 8 + C], f32)
            xpad = load(bigp, d_xpad, [C, HP, WP], bf)
            ldiag = load(bigp, d_ldiag, [C, K2, C], bf)

            wkT = wpack[:, 0:C]
            wq = wpack[:, C:2 * C]
            wvfT = wpack[:, 2 * C:3 * C]
            wgT = wpack[:, 3 * C:4 * C]
            ident = wpack[:, 4 * C:5 * C]
            wwT = wpack[:K2, 5 * C:5 * C + K2]
            ones49 = wpack[:K2, 5 * C + K2:5 * C + K2 + 1]
            ones1_49 = wpack[:1, 5 * C + K2 + 1:5 * C + 2 * K2 + 1]
            bqcol = wpack[:, 5 * C + 2 * K2 + 1:5 * C + 2 * K2 + 2]

            bvfcol = fpack[:, 0:1]
            bkcol = fpack[:, 1:2]
            bfcol = fpack[:, 2:3]
            gbs = fpack[:, 3:4]
            gbb = fpack[:, 4:5]
            bwcol = fpack[:K2, 5:6]
            caw1T = fpack[:, 6:14]
            caw2T = fpack[:8, 14:14 + C]

            # persistent lepe accumulator: 4 chunks x 1 bank
            lp = lepep.tile([C, NCHUNK, 512], f32, tag="lp")

            # lepe diag matmul emitter (tap-major; interleaved as PE filler)
            lepe_iter = iter(range(K2))

            def emit_lepe(n_taps):
                for _ in range(n_taps):
                    k = next(lepe_iter, None)
                    if k is None:
                        return
                    kh, kw = divmod(k, K)
                    for ci in range(NCHUNK):
                        rhs = xpad[:, kh + ci * 7:kh + ci * 7 + 7, kw:kw + W]
                        nc.tensor.matmul(
                            lp[:, ci, :CH_N], lhsT=ldiag[:, k, :], rhs=rhs,
                            start=(k == 0), stop=False,
                        )

            # ============ phase A ============
            with (
                tc.tile_pool(name="psA", bufs=1, space="PSUM") as psA,
                tc.tile_pool(name="mmA", bufs=2, space="PSUM") as mmA,
            ):
                # pooled key: x_up [C,56,56] -> block sum 8x8 -> [C,7,7]
                pool1 = bigp.tile([C, H, 7], f32)
                nc.vector.tensor_reduce(
                    out=pool1[:],
                    in_=xup[:].rearrange("p h (bw dw) -> p h bw dw", dw=8),
                    axis=AX.X, op=ALU.add,
                )
                pooledf = workp.tile([C, 7, 7], f32, tag="pooledf")
                nc.vector.tensor_reduce(
                    out=pooledf[:],
                    in_=pool1[:].rearrange("p (bh dh) bw -> p bh bw dh", dh=8),
                    axis=AX.X, op=ALU.add,
                )
                pooled = workp.tile([C, K2], bf, tag="pooled")
                nc.vector.tensor_copy(pooled[:], pooledf[:].rearrange("p a b -> p (a b)"))

                # key = wk @ pooled + bk   (wk pre-divided by 64 on host)
                key_ps = mmA.tile([C, 512], f32, tag="mm")
                nc.tensor.matmul(key_ps[:, :K2], lhsT=wkT, rhs=pooled[:], start=True, stop=True)
                key = workp.tile([C, K2], bf, tag="key")
                nc.scalar.activation(key[:], key_ps[:, :K2], AF.Identity, bias=bkcol, scale=1.0)

                # eb = SCALE * key^T bq  -> [49,1] exp bias
                eb_ps = mmA.tile([K2, 512], f32, tag="mm")
                nc.tensor.matmul(eb_ps[:, :1], lhsT=key[:], rhs=bqcol, start=True, stop=True)
                eb = constp.tile([K2, 1], f32)
                nc.scalar.mul(eb[:], eb_ps[:, :1], SCALE)

                # KW = wq^T @ key  [C, 49]
                kw_ps = mmA.tile([C, 512], f32, tag="mm")
                nc.tensor.matmul(kw_ps[:, :K2], lhsT=wq, rhs=key[:], start=True, stop=True)
                KW = workp.tile([C, K2], bf, tag="KW")
                nc.scalar.copy(KW[:], kw_ps[:, :K2])

                # scores -> exp  (scores = KW^T @ x_skip) -- the dynw critical
                # chain runs lepe-free so the tap loop can start ASAP
                expv = bigp.tile([K2, N], bf)
                for hi in range(2):
                    sc_ps = psA.tile([K2, 2, 512], f32, tag="big49")
                    for cj in range(2):
                        ci = hi * 2 + cj
                        rhs = xpad[:, 3 + ci * 7:3 + ci * 7 + 7, 3:3 + W]
                        nc.tensor.matmul(sc_ps[:, cj, :CH_N], lhsT=KW[:], rhs=rhs, start=True, stop=True)
                    nc.scalar.activation(
                        expv[:, hi * HN:(hi + 1) * HN].rearrange("p (a x) -> p a x", a=2),
                        sc_ps[:, :, :CH_N], AF.Exp, bias=eb[:], scale=SCALE,
                    )

                # Z row sums -> 1/Z (fast approx reciprocal on the idle
                # prefix DVE; bw joins dynw additively after normalization)
                zsb = workp.tile([1, N], f32, tag="zsb")
                for ci in range(NCHUNK):
                    z_ps = mmA.tile([1, 512], f32, tag="mm")
                    sl = slice(ci * CH_N, (ci + 1) * CH_N)
                    nc.tensor.matmul(z_ps[:, :CH_N], lhsT=ones49, rhs=expv[:, sl], start=True, stop=True)
                    nc.scalar.copy(zsb[:, sl], z_ps[:, :CH_N])
                rec_rowf = workp.tile([1, N], f32, tag="rec_rowf")
                nc.vector.reciprocal_approx_fast(rec_rowf[:], zsb[:])
                rec_row = workp.tile([1, N], bf, tag="rec_row")
                nc.vector.tensor_copy(rec_row[:], rec_rowf[:])

                # recbc = broadcast 1/Z to 49 partitions; dynw = (wwT@exp + bw x Z) * recbc
                dynw = bigp.tile([K2, N], bf)
                recbc = bigp.tile([K2, N], bf)
                for hi in range(2):
                    rb_ps = psA.tile([K2, 2, 512], f32, tag="big49")
                    for cj in range(2):
                        ci = hi * 2 + cj
                        sl = slice(ci * CH_N, (ci + 1) * CH_N)
                        nc.tensor.matmul(rb_ps[:, cj, :CH_N], lhsT=ones1_49, rhs=rec_row[:, sl], start=True, stop=True)
                    nc.scalar.copy(
                        recbc[:, hi * HN:(hi + 1) * HN].rearrange("p (a x) -> p a x", a=2),
                        rb_ps[:, :, :CH_N],
                    )
                for hi in range(2):
                    dU_ps = psA.tile([K2, 2, 512], f32, tag="big49")
                    for cj in range(2):
                        ci = hi * 2 + cj
                        sl = slice(ci * CH_N, (ci + 1) * CH_N)
                        nc.tensor.matmul(dU_ps[:, cj, :CH_N], lhsT=wwT, rhs=expv[:, sl], start=True, stop=True)
                    hsl = slice(hi * HN, (hi + 1) * HN)
                    nc.vector.tensor_mul(
                        dynw[:, hsl].rearrange("p (a x) -> p a x", a=2),
                        dU_ps[:, :, :CH_N],
                        recbc[:, hsl].rearrange("p (a x) -> p a x", a=2),
                    )
                # dyn_w = ww@attn + bw  (bw joins after the 1/Z normalization)
                nc.vector.tensor_scalar_add(dynw[:], dynw[:], bwcol)

                # ship dynw to DRAM for the per-tap replicate DMAs
                nc.sync.dma_start(out=d_dynw[:], in_=dynw[:])

                # V' = (fus_w @ wv) @ x_skip, zero-padded borders
                vpad = bigp.tile([C, HP, WP], bf)
                nc.vector.memset(vpad[:], 0.0)
                for ci in range(5):  # 34 rows: 7,7,7,7,6
                    r = ci * 7
                    nr = min(7, HP - r)
                    v_ps = mmA.tile([C, 512], f32, tag="mm")
                    rhs = xpad[:, r:r + nr, 3:3 + W]
                    nc.tensor.matmul(v_ps[:, :nr * W], lhsT=wvfT, rhs=rhs, start=True, stop=True)
                    nc.scalar.activation(
                        vpad[:, r:r + nr, 3:3 + W], v_ps[:, :nr * W], AF.Identity,
                        bias=bvfcol, scale=1.0,
                    )
                    emit_lepe(1)

                # gate = silu(bn(wg @ x_skip)) in bf16
                gate = bigp.tile([C, N], bf)
                for ci in range(NCHUNK):
                    g_ps = mmA.tile([C, 512], f32, tag="mm")
                    rhs = xpad[:, 3 + ci * 7:3 + ci * 7 + 7, 3:3 + W]
                    nc.tensor.matmul(g_ps[:, :CH_N], lhsT=wgT, rhs=rhs, start=True, stop=True)
                    nc.scalar.activation(
                        gate[:, ci * CH_N:(ci + 1) * CH_N], g_ps[:, :CH_N], AF.Silu,
                        bias=gbb, scale=gbs,
                    )
                    emit_lepe(1)

            # ============ phase B: 49-tap dynamic conv + lepe filler ============
            # Odd taps >=5 send their product to the (otherwise idle) PE,
            # which accumulates them into the lepe PSUM via identity matmuls;
            # even taps accumulate on the DVE in bf16 chains.
            PE_ADD = frozenset(k for k in range(5, K2) if k % 2 == 1)
            acc = [bigp.tile([C, N], bf, name=f"acc{j}", tag=f"acc{j}") for j in range(4)]
            dve_first = [True] * 4
            dma_engines = [nc.sync, nc.scalar]
            TG = 4  # taps per replicate DMA (amortizes per-packet overhead)

            def pe_accumulate(tmp, stop):
                for ci in range(NCHUNK):
                    nc.tensor.matmul(
                        lp[:, ci, :CH_N], lhsT=ident,
                        rhs=tmp[:, ci * 7:(ci + 1) * 7, :],
                        start=False, stop=stop,
                    )

            tmp47 = None
            tap_groups = [list(range(g, min(g + TG, K2))) for g in range(0, K2, TG)]
            for gi, grp in enumerate(tap_groups):
                # broadcast dynw rows grp to all 128 partitions via stride-0
                # DMA, alternating between the two HWDGE queues
                bcb = pipep.tile([C, TG, HALF, W], bf, tag="bcb", bufs=3)
                nt = len(grp)
                dma_engines[gi % 2].dma_start(
                    out=bcb[:, :nt, :, :],
                    in_=d_dynw[grp[0]:grp[0] + nt, :].rearrange(
                        "t (r w) -> t r w", w=W).partition_broadcast(C))
                for k in grp:
                    kh, kw = divmod(k, K)
                    vap = vpad[:, kh:kh + HALF, kw:kw + W]
                    bck = bcb[:, k - grp[0], :, :]
                    if k in PE_ADD:
                        if k == 47:
                            tmp47 = pipep.tile([C, HALF, W], bf, tag="tmp47", bufs=1)
                            nc.vector.tensor_mul(tmp47[:], vap, bck)
                        else:
                            tmp = pipep.tile([C, HALF, W], bf, tag="ptmp", bufs=3)
                            nc.vector.tensor_mul(tmp[:], vap, bck)
                            pe_accumulate(tmp[:], stop=False)
                    else:
                        j = k % 4
                        tgt = acc[j]
                        first = dve_first[j]
                        dve_first[j] = False
                        osl = tgt[:].rearrange("p (r w) -> p r w", w=W)
                        if first:
                            nc.vector.tensor_mul(osl, vap, bck)
                        else:
                            tmp = pipep.tile([C, HALF, W], bf, tag="dtmp")
                            nc.vector.tensor_mul(tmp[:], vap, bck)
                            nc.vector.tensor_add(osl, osl, tmp[:])
                    emit_lepe(1)

            emit_lepe(K2)  # drain any remaining lepe taps
            # tap 47 closes the lepe/PE-add accumulation group
            pe_accumulate(tmp47[:], stop=True)

            # ============ tail ============
            xid = load(bigp, d_xid, [C, N], f32)
            a01 = bigp.tile([C, N], bf)
            nc.vector.tensor_add(a01[:], acc[0][:], acc[1][:])
            a23 = bigp.tile([C, N], bf)
            nc.vector.tensor_add(a23[:], acc[2][:], acc[3][:])
            pre = bigp.tile([C, N], bf)
            nc.vector.tensor_add(pre[:], a01[:], a23[:])

            xf = bigp.tile([C, N], bf)
            scol = workp.tile([C, NCHUNK], f32, tag="scol")
            for ci in range(NCHUNK):
                sl = slice(ci * CH_N, (ci + 1) * CH_N)
                nc.vector.scalar_tensor_tensor(
                    out=xf[:, sl], in0=pre[:, sl], scalar=bfcol, in1=lp[:, ci, :CH_N],
                    op0=ALU.add, op1=ALU.add, accum_out=scol[:, ci:ci + 1],
                )

            g2 = bigp.tile([C, N], bf)
            nc.vector.tensor_mul(g2[:], gate[:], xf[:])

            # channel stats
            mx = workp.tile([C, 1], f32, tag="mx")
            nc.vector.tensor_reduce(out=mx[:], in_=xf[:], axis=AX.X, op=ALU.max)
            s01 = workp.tile([C, 1], f32, tag="s01")
            nc.vector.tensor_add(s01[:], scol[:, 0:1], scol[:, 1:2])
            s23 = workp.tile([C, 1], f32, tag="s23")
            nc.vector.tensor_add(s23[:], scol[:, 2:3], scol[:, 3:4])
            stats = workp.tile([C, 2], f32, tag="stats")
            nc.vector.tensor_add(stats[:, 0:1], s01[:], s23[:])
            nc.vector.tensor_copy(stats[:, 1:2], mx[:])

            stat4 = workp.tile([C, 4], f32, tag="stat4")
            if use_collectives:
                nc.sync.dma_start(out=cc_in[:], in_=stats[:])
                nc.gpsimd.collective_compute(
                    "AllGather", ALU.bypass, replica_groups=groups,
                    ins=[cc_in[:]], outs=[cc_out[:]],
                )
                nc.sync.dma_start(
                    out=stat4[:].rearrange("c (r s) -> c r s", r=2),
                    in_=cc_out[:].rearrange("(r c) s -> c r s", r=2))
            else:
                nc.vector.tensor_copy(stat4[:, 0:2], stats[:])
                nc.vector.tensor_copy(stat4[:, 2:4], stats[:])

            ca_in = workp.tile([C, 2], f32, tag="ca_in")
            nc.vector.tensor_add(ca_in[:, 0:1], stat4[:, 0:1], stat4[:, 2:3])
            nc.vector.tensor_scalar_mul(ca_in[:, 0:1], ca_in[:, 0:1], 1.0 / (H * W))
            nc.vector.tensor_tensor(
                out=ca_in[:, 1:2], in0=stat4[:, 1:2], in1=stat4[:, 3:4], op=ALU.max)

            with tc.tile_pool(name="mmC", bufs=2, space="PSUM") as mmC:
                r1_ps = mmC.tile([8, 512], f32, tag="mm")
                nc.tensor.matmul(r1_ps[:, :2], lhsT=caw1T, rhs=ca_in[:], start=True, stop=True)
                r1 = workp.tile([8, 2], f32, tag="r1")
                nc.scalar.activation(r1[:], r1_ps[:, :2], AF.Relu, bias=0.0, scale=1.0)
                r2_ps = mmC.tile([C, 512], f32, tag="mm")
                nc.tensor.matmul(r2_ps[:, :2], lhsT=caw2T, rhs=r1[:], start=True, stop=True)
                r2sb = workp.tile([C, 2], f32, tag="r2sb")
                nc.vector.tensor_copy(r2sb[:], r2_ps[:, :2])
                casum = workp.tile([C, 1], f32, tag="casum")
                nc.vector.tensor_add(casum[:], r2sb[:, 0:1], r2sb[:, 1:2])
                ca = workp.tile([C, 1], f32, tag="ca")
                nc.scalar.activation(ca[:], casum[:], AF.Sigmoid, bias=0.0, scale=1.0)

            outt = bigp.tile([C, N], f32)
            nc.vector.scalar_tensor_tensor(
                out=outt[:], in0=g2[:], scalar=ca[:], in1=xid[:],
                op0=ALU.mult, op1=ALU.add,
            )
            nc.sync.dma_start(out=d_y[:], in_=outt[:])

    nc.compile()
    return nc


def _host_prep(inputs):
    """Build per-core input maps (slicing / dtype casts / small weight folds)."""
    f = {k: np.asarray(v, dtype=np.float32) for k, v in inputs.items()}

    # lepe reparam: fold 5 depthwise convs + BNs into one 7x7 kernel + bias
    w7 = f["lk_w"][:, 0] * f["lk_bn_s"][:, None, None]
    w7[:, 1:6, 1:6] += f["dw5"][:, 0] * f["bn5_s"][:, None, None]
    w7[:, 2:5, 2:5] += f["dw3a"][:, 0] * f["bn3a_s"][:, None, None]
    w7[:, 1::2, 1::2] += f["dw3b"][:, 0] * f["bn3b_s"][:, None, None]
    w7[:, ::3, ::3] += f["dw3c"][:, 0] * f["bn3c_s"][:, None, None]
    W_eff = f["lepe_bn_s"][:, None, None] * w7
    b_eff = (
        f["lepe_bn_s"]
        * (f["lk_bn_b"] + f["bn5_b"] + f["bn3a_b"] + f["bn3b_b"] + f["bn3c_b"])
        + f["lepe_bn_b"]
    )
    bias_fused = (b_eff + f["fus_b"]).astype(np.float32)

    ldiag = np.zeros((C, K2, C), dtype=BF16)
    cc = np.arange(C)
    for k in range(K2):
        ldiag[cc, k, cc] = W_eff[:, k // K, k % K].astype(BF16)

    wvf = f["fus_w"] @ f["wv"]           # folded value conv
    bvf = f["fus_w"] @ f["bv"]           # its bias (applied inside vpad interior)

    # bf16 pack: wkT | wq | wvfT | wgT | ident | wwT | ones49 | ones1_49 | bq
    wpack = np.zeros((C, 5 * C + 2 * K2 + 2), dtype=BF16)
    wpack[:, 0:C] = (f["wk"] / 64.0).T.astype(BF16)
    wpack[:, C:2 * C] = f["wq"].astype(BF16)
    wpack[:, 2 * C:3 * C] = wvf.T.astype(BF16)
    wpack[:, 3 * C:4 * C] = f["gate_w"].T.astype(BF16)
    wpack[:, 4 * C:5 * C] = np.eye(C, dtype=BF16)
    wpack[:K2, 5 * C:5 * C + K2] = f["ww"].T.astype(BF16)
    wpack[:K2, 5 * C + K2] = 1.0
    wpack[0, 5 * C + K2 + 1:5 * C + 2 * K2 + 1] = 1.0
    wpack[:, 5 * C + 2 * K2 + 1] = f["bq"].astype(BF16)

    # f32 pack: bvf | bk | biasf | gbs | gbb | bw | caw1T | caw2T
    fpack = np.zeros((C, 6 + 8 + C), dtype=np.float32)
    fpack[:, 0] = bvf
    fpack[:, 1] = f["bk"]
    fpack[:, 2] = bias_fused
    fpack[:, 3] = f["gate_bn_s"]
    fpack[:, 4] = f["gate_bn_b"]
    fpack[:K2, 5] = f["bw"]
    fpack[:, 6:14] = f["ca_w1"].T
    fpack[:8, 14:14 + C] = f["ca_w2"].T

    common = {
        "lepe_diag": ldiag,
        "wpack": wpack,
        "fpack": fpack,
    }

    xsk_pad = np.zeros((B, C, H + 6, W + 6), dtype=np.float32)
    xsk_pad[:, :, 3:3 + H, 3:3 + W] = f["x_skip"]
    rs = float(f["res_scale"][0])

    in_maps = []
    for core in range(8):
        b, half = divmod(core, 2)
        r0 = half * HALF
        m = dict(common)
        m["x_pad"] = xsk_pad[b, :, r0:r0 + HP, :].astype(BF16)
        m["x_id"] = (f["x_skip"][b, :, r0:r0 + HALF, :].reshape(C, N) * rs).astype(np.float32)
        m["x_up"] = f["x_up"][b].astype(BF16)
        in_maps.append(m)
    return in_maps


def kernel(**inputs):
    if "nc" not in _CACHE:
        _CACHE["nc"] = _build_program()
    nc = _CACHE["nc"]
    in_maps = _host_prep(inputs)
    res = run_bass_kernel_spmd(nc, in_maps, list(range(8)))
    out = np.empty((B, C, H, W), dtype=np.float32)
    for core in range(8):
        b, half = divmod(core, 2)
        r0 = half * HALF
        out[b, :, r0:r0 + HALF, :] = res.results[core]["y"].reshape(C, HALF, W)
    return out
